# revision 28
# baseline (speedup 1.0000x reference)
"""Trainium2 Bass kernel for an 8-expert MoE FFN layer (nn_MoELayer).

Reference computation (per expert e over its contiguous 1024-token chunk):
    h = gelu(x_e @ w1[e] + b1[e]);  y_e = h @ w2[e] + b2[e]

Sharding: expert parallelism — core e holds expert e's weights and its token
chunk (the gate yields equal contiguous chunks, so no all-to-all is needed).
Each core runs the same SPMD program on its own data.

Per-core work (T=1024 tokens, D=1024, F=4096), all matmuls fp16 with fp32
PSUM accumulation (~216 ns per 512-wide matmul, the PE floor).  BOTH GEMMs
run one level of Strassen-Winograd (7 products instead of 8 over the
[out/2, contract/2, tokens/2] block split), cutting PE work to 2x448 matmuls
instead of 2x512:

  phase 1 (h = gelu(w1^T x + b1)): A-side (w1) and B-side (x) combos are
  prepared on the host.  Per unit i (f-tile rows i and 16+i): products
  M1..M7 on the PE, C-assembly U-chain on the vector engine, M1 evacuation
  and the four gelu+bias on the scalar engine, phase-2 B-side combos of h
  on the gpsimd + vector engines — all overlapped with the PE.

  phase 2 (y = w2^T h + b2): A-side (w2) combos on the host; B-side combos
  T1..T4 of h built during phase 1; C-side assembly on the vector engine
  with the b2 bias folded into fused scalar_tensor_tensor ops.

Head: the matmul stream is gated by ~0.6 MiB (first w1 slab + x chunk-0
lower half) on a latency-bound (~200-300 GB/s) early DMA path; the critical
pieces stream on the sync/SP HWDGE ring in product order, and dummy warmup
matmuls on scratch bridge the PE from the preamble barrier (also opening the
HAM clock-warmup window).  Unit 0 is wire-paced; units 1+ reuse the resident
x operands and stream only weight slabs.

Tail: the last product of the last phase-2 unit runs as two 256-wide PSUM
groups so only a 256-column assembly+flush remains after the final matmul.
"""

import os

import numpy as np

# The kernel executes through the axon PJRT backend; a CPU pin (e.g. set for
# a jax reference run) would break NEFF dispatch in this process.
if os.environ.get("JAX_PLATFORMS") == "cpu":
    del os.environ["JAX_PLATFORMS"]

E = 8          # experts == cores
B, S = 2, 4096
D = 1024       # d_model
F = 4096       # d_ff
T = (B * S) // E  # tokens per expert chunk = 1024
P = 128
DO = D // P    # 8  k-tiles of d_model
FT = F // P    # 32 f-tiles of d_ff
DMO = D // P   # 8  output dm-tiles
FH = FT // 2   # 16 f-tiles per Strassen half of d_ff
DH = DO // 2   # 4  d-model k-tiles per Strassen half
NCHUNK = T // 512
N_WARMUP_MM = 31

_cached = None


def _build():
    import concourse.mybir as mybir
    import concourse.tile as tile
    from concourse import bacc
    from concourse.tile_rust import add_dep_helper

    f32 = mybir.dt.float32
    f16 = mybir.dt.float16
    add = mybir.AluOpType.add
    sub = mybir.AluOpType.subtract

    nc = bacc.Bacc("TRN2", target_bir_lowering=False, debug=False, num_devices=E)

    # x: chunk 0 fully; of chunk 1 only the d-bottom half (B22) is read on
    # device — B12 only enters via the host-built combos.
    xT_d = nc.dram_tensor("xT", [NCHUNK, P, DO, 512], f16, kind="ExternalInput")
    # host-built Strassen B-side combos of x: [combo(T1..T4), p, k, t]
    xS_d = nc.dram_tensor("xS", [4, P, DH, 512], f16, kind="ExternalInput")
    # Strassen A-side operands of w1, paired along the product order so
    # each DMA is 256 KiB with 2 KiB descriptors: groups (M1,M2) (M6,M7)
    # (M5,M3) in w1p, M4 solo in w1q.  [g, unit, p(d), pairidx, k, f]
    w1p_d = nc.dram_tensor("w1p", [3, FH, P, 2, DH, P], f16, kind="ExternalInput")
    w1q_d = nc.dram_tensor("w1q", [FH, P, DH, P], f16, kind="ExternalInput")
    bc_d = nc.dram_tensor("bc", [P, FT + DMO], f32, kind="ExternalInput")
    # Strassen A-side operands of w2: [mi, j, p(f-within-half), k, dm]
    w2_d = nc.dram_tensor("w2s", [7, 4, P, FH, P], f16, kind="ExternalInput")
    yT_d = nc.dram_tensor("yT", [DMO, P, T], f32, kind="ExternalOutput")

    gelu = mybir.ActivationFunctionType.Gelu_apprx_tanh
    act_copy = mybir.ActivationFunctionType.Copy

    # product order: consume-as-completed C-assembly with each PSUM bank
    # released right after its single consumer op
    COMBO = {4: 0, 5: 1, 6: 2, 3: 3}  # mi -> T-combo row (M5>T1 M6>T2 M7>T3 M4>T4)

    with tile.TileContext(nc) as tc:
        with (
            tc.tile_pool(name="xpool", bufs=1) as xpool,
            tc.tile_pool(name="hpool", bufs=1) as hpool,
            tc.tile_pool(name="tpool", bufs=1) as tpool,
            tc.tile_pool(name="wpool", bufs=2) as wpool,
            tc.tile_pool(name="cpool", bufs=1) as cpool,
            tc.tile_pool(name="spool", bufs=2) as spool,
            tc.tile_pool(name="ypool", bufs=2) as ypool,
            tc.tile_pool(name="psum_h", bufs=2, space="PSUM") as psum_h,
            tc.tile_pool(name="psum_y", bufs=2, space="PSUM") as psum_y,
        ):
            # scratch for PE warmup: direct f16 memset on the vector engine
            scratch = cpool.tile([P, 512], f16)
            nc.vector.memset(scratch[:], 0.0)

            # SBUF residency: x chunk 0 (8 k-tiles) + x chunk-1 d-bottom
            xT_sb = xpool.tile([P, 12, 512], f16)   # [c0 k0..7 | c1b k0..3]
            xS_sb = xpool.tile([P, 4, DH, 512], f16)
            xc0 = xT_d.ap()[0].rearrange("p do t -> p (do t)")
            xc1 = xT_d.ap()[1].rearrange("p do t -> p (do t)")
            QX = DO * 512 // 4

            def slab_pair(g, i):
                s = wpool.tile([P, 2, DH, P], f16, tag="w1a", bufs=4,
                               name="w1p_sb")
                nc.sync.dma_start(s[:], w1p_d.ap()[g, i])
                return s

            def slab_solo(i):
                s = wpool.tile([P, DH, P], f16, tag="w1b", bufs=2, name="w1q_sb")
                nc.sync.dma_start(s[:], w1q_d.ap()[i])
                return s

            bc_sb = cpool.tile([P, FT + DMO], f32)
            # ---- head DMAs on the sync ring, in unit-0 consumption order
            u0_pairs = {}
            u0_pairs[0] = slab_pair(0, 0)
            for q in range(4):
                nc.sync.dma_start(
                    xT_sb[:, 2 * q : 2 * q + 2, :].rearrange("p a b -> p (a b)"),
                    xc0[:, q * QX : (q + 1) * QX],
                )
            nc.gpsimd.dma_start(bc_sb[:], bc_d.ap())
            u0_pairs[1] = slab_pair(1, 0)
            nc.sync.dma_start(xS_sb[:, 1], xS_d.ap()[1])  # T2
            nc.sync.dma_start(xS_sb[:, 2], xS_d.ap()[2])  # T3
            u0_pairs[2] = slab_pair(2, 0)
            nc.sync.dma_start(xS_sb[:, 0], xS_d.ap()[0])  # T1
            nc.sync.dma_start(  # B22 = x chunk-1 d-bottom
                xT_sb[:, 8:12, :].rearrange("p a b -> p (a b)"),
                xc1[:, 4 * 512 :],
            )
            u0_solo = slab_solo(0)
            nc.sync.dma_start(xS_sb[:, 3], xS_d.ap()[3])  # T4
            b1_sb = bc_sb[:, :FT]
            b2_sb = bc_sb[:, FT:]

            # PE warmup while the head DMAs stream
            for i in range(N_WARMUP_MM):
                pw = psum_y.tile([P, 512], f32, tag="pm", bufs=4, name="pwarm")
                nc.tensor.matmul(
                    pw[:], scratch[:, :P], scratch[:], start=True, stop=True
                )

            h_sb = hpool.tile([P, FT, T], f16)
            t_sb = tpool.tile([P, 4, FH, 512], f16)

            def x_moving(mi, k):
                if mi == 0:
                    return xT_sb[:, k, :]        # B11
                if mi == 1:
                    return xT_sb[:, 4 + k, :]    # B21
                if mi == 2:
                    return xT_sb[:, 8 + k, :]    # B22
                return xS_sb[:, COMBO[mi], k, :]

            PAIR_OF = {0: (0, 0), 1: (0, 1), 5: (1, 0), 6: (1, 1),
                       4: (2, 0), 2: (2, 1)}  # mi -> (group, pair index)
            live_pairs = {}

            def p1_product(mi, i):
                pm = psum_h.tile([P, 512], f32, tag="ph", bufs=4, name="ph")
                if mi == 3:
                    s = u0_solo if i == 0 else slab_solo(i)
                    st = lambda k: s[:, k, :]
                else:
                    g, idx = PAIR_OF[mi]
                    if idx == 0:
                        live_pairs[g] = u0_pairs[g] if i == 0 else slab_pair(g, i)
                    s = live_pairs[g]
                    st = lambda k: s[:, idx, k, :]
                for k in range(DH):
                    nc.tensor.matmul(
                        pm[:], st(k), x_moving(mi, k),
                        start=(k == 0), stop=(k == DH - 1),
                    )
                return pm

            # ---- phase 1: per unit i, products M1..M7 then Winograd
            # C-assembly: C11=M1+M2 C12=U4+M3 C21=U3-M4 C22=U3+M5,
            # U2=M1+M6 U3=U2+M7 U4=U2+M5; gelu+b1 on the scalar engine.
            gelu_insts = {}
            for i in range(FH):
                b1i = b1_sb[:, i : i + 1]
                b1i2 = b1_sb[:, FH + i : FH + i + 1]

                def gelu_to(src, ft, c):
                    cs = slice(c * 512, (c + 1) * 512)
                    gelu_insts[(ft, c)] = nc.scalar.activation(
                        h_sb[:, ft, cs], src, gelu, bias=(b1i if ft < FH else b1i2)
                    )

                pm1 = p1_product(0, i)
                s_m1 = spool.tile([P, 512], f32, tag="sm1", bufs=2, name="s_m1")
                nc.scalar.activation(s_m1[:], pm1[:], act_copy, bias=0.0)
                pm2 = p1_product(1, i)
                u1 = spool.tile([P, 512], f32, tag="ug", bufs=2, name="u1")
                nc.vector.tensor_add(u1[:], s_m1[:], pm2[:])
                gelu_to(u1[:], i, 0)                      # C11
                pm6 = p1_product(5, i)
                u2 = spool.tile([P, 512], f32, tag="u2", bufs=2, name="u2")
                nc.vector.tensor_add(u2[:], s_m1[:], pm6[:])
                pm7 = p1_product(6, i)
                u3 = spool.tile([P, 512], f32, tag="u3", bufs=2, name="u3")
                nc.vector.tensor_add(u3[:], u2[:], pm7[:])
                pm5 = p1_product(4, i)
                u4 = spool.tile([P, 512], f32, tag="u4", bufs=2, name="u4")
                nc.vector.tensor_add(u4[:], u2[:], pm5[:])
                u7 = spool.tile([P, 512], f32, tag="ug", bufs=2, name="u7")
                nc.vector.tensor_add(u7[:], u3[:], pm5[:])
                gelu_to(u7[:], FH + i, 1)                 # C22
                pm3 = p1_product(2, i)
                u5 = spool.tile([P, 512], f32, tag="ug", bufs=2, name="u5")
                nc.vector.tensor_add(u5[:], u4[:], pm3[:])
                gelu_to(u5[:], i, 1)                      # C12
                pm4 = p1_product(3, i)
                u6 = spool.tile([P, 512], f32, tag="ug", bufs=2, name="u6")
                nc.vector.tensor_sub(u6[:], u3[:], pm4[:])
                gelu_to(u6[:], FH + i, 0)                 # C21

                # phase-2 B-side combos of h for this unit: T1/T3 on the
                # idle gpsimd engine, the chained T2/T4 on the vector engine
                nc.gpsimd.tensor_sub(
                    t_sb[:, 0, i, :], h_sb[:, i, 512:1024], h_sb[:, i, 0:512]
                )
                nc.gpsimd.tensor_sub(
                    t_sb[:, 2, i, :], h_sb[:, FH + i, 512:1024],
                    h_sb[:, i, 512:1024],
                )
                nc.gpsimd.tensor_sub(
                    t_sb[:, 1, i, :], h_sb[:, FH + i, 512:1024], t_sb[:, 0, i, :]
                )
                nc.gpsimd.tensor_sub(
                    t_sb[:, 3, i, :], t_sb[:, 1, i, :], h_sb[:, FH + i, 0:512]
                )

            # ---- phase 2: y = w2^T h + b2, Strassen-Winograd (as phase 1,
            # with 16 k-tiles per product and the b2 bias folded into the
            # final fused ops)
            def h_moving(mi, k):
                if mi == 0:
                    return h_sb[:, k, 0:512]
                if mi == 1:
                    return h_sb[:, FH + k, 0:512]
                if mi == 2:
                    return h_sb[:, FH + k, 512:1024]
                return t_sb[:, COMBO[mi], k, :]

            def p2_product(mi, j, n0=0, n1=512, pm=None):
                if pm is None:
                    pm_t = psum_y.tile([P, 512], f32, tag="pm", bufs=4, name="pm")
                    pm = pm_t[:, n0:n1]
                for half in range(2):
                    slab = wpool.tile([P, FH // 2, P], f16, tag="w2a", bufs=7,
                                      name="w2s_sb")
                    dma = nc.sync.dma_start(
                        slab[:],
                        w2_d.ap()[mi, j][:, half * 8 : (half + 1) * 8, :],
                    )
                    if mi == 0 and j == 0:
                        add_dep_helper(
                            dma.ins,
                            gelu_insts[(6, 1)].ins,
                            sync=True,
                            reason="delay w2 prefetch past the kernel head",
                        )
                    for kk in range(FH // 2):
                        k = half * 8 + kk
                        nc.tensor.matmul(
                            pm,
                            slab[:, kk, :],
                            h_moving(mi, k)[:, n0:n1],
                            start=(k == 0),
                            stop=(k == FH - 1),
                        )
                return pm

            def flush(src_sb, j_out, cs):
                nc.sync.dma_start(yT_d.ap()[j_out][:, cs], src_sb[:])

            for j in range(4):
                b2j = b2_sb[:, j : j + 1]
                b2j4 = b2_sb[:, 4 + j : 5 + j]
                pm1 = p2_product(0, j)
                s_m1 = spool.tile([P, 512], f32, tag="sm1", bufs=2, name="s_m1")
                nc.vector.tensor_copy(s_m1[:], pm1)
                pm2 = p2_product(1, j)
                y11 = ypool.tile([P, 512], f32, tag="y", bufs=2, name="y_sb")
                nc.vector.scalar_tensor_tensor(y11[:], s_m1[:], b2j, pm2, add, add)
                flush(y11, j, slice(0, 512))
                pm6 = p2_product(5, j)
                u2 = spool.tile([P, 512], f32, tag="u2", bufs=2, name="u2")
                nc.vector.tensor_add(u2[:], s_m1[:], pm6)
                pm7 = p2_product(6, j)
                u3 = spool.tile([P, 512], f32, tag="u3", bufs=2, name="u3")
                nc.vector.tensor_add(u3[:], u2[:], pm7)
                pm5 = p2_product(4, j)
                u4 = spool.tile([P, 512], f32, tag="u4", bufs=2, name="u4")
                nc.vector.tensor_add(u4[:], u2[:], pm5)
                y22 = ypool.tile([P, 512], f32, tag="y", bufs=2, name="y_sb")
                nc.vector.scalar_tensor_tensor(y22[:], u3[:], b2j4, pm5, add, add)
                flush(y22, 4 + j, slice(512, 1024))
                pm3 = p2_product(2, j)
                y12 = ypool.tile([P, 512], f32, tag="y", bufs=2, name="y_sb")
                nc.vector.scalar_tensor_tensor(y12[:], u4[:], b2j, pm3, add, add)
                flush(y12, j, slice(512, 1024))
                if j < 3:
                    pm4 = p2_product(3, j)
                    y21 = ypool.tile([P, 512], f32, tag="y", bufs=2, name="y_sb")
                    nc.vector.scalar_tensor_tensor(
                        y21[:], u3[:], b2j4, pm4, add, sub
                    )
                    flush(y21, 4 + j, slice(0, 512))
                else:
                    # last unit: M4 as two 256-wide groups so only a 256-col
                    # assembly+flush remains after the final matmul
                    pm4a_t = psum_y.tile([P, 512], f32, tag="pm", bufs=4,
                                         name="pm4a")
                    pm4a = pm4a_t[:, 0:256]
                    p2_product(3, j, 0, 256, pm4a)
                    y21a = ypool.tile([P, 256], f32, tag="yh", bufs=2, name="y21a")
                    nc.vector.scalar_tensor_tensor(
                        y21a[:], u3[:, 0:256], b2j4, pm4a, add, sub
                    )
                    flush(y21a, 4 + j, slice(0, 256))
                    pm4b_t = psum_y.tile([P, 512], f32, tag="pm", bufs=4,
                                         name="pm4b")
                    pm4b = pm4b_t[:, 0:256]
                    p2_product(3, j, 256, 512, pm4b)
                    y21b = ypool.tile([P, 256], f32, tag="yh", bufs=2, name="y21b")
                    nc.vector.scalar_tensor_tensor(
                        y21b[:], u3[:, 256:512], b2j4, pm4b, add, sub
                    )
                    nc.scalar.dma_start(yT_d.ap()[4 + j][:, 256:512], y21b[:])

    nc.compile()
    return nc


def _get_nc():
    global _cached
    if _cached is None:
        _cached = _build()
    return _cached


def make_in_maps(x, w1, b1, w2, b2):
    x = np.asarray(x, dtype=np.float32)
    w1 = np.asarray(w1, dtype=np.float32)
    b1 = np.asarray(b1, dtype=np.float32)
    w2 = np.asarray(w2, dtype=np.float32)
    b2 = np.asarray(b2, dtype=np.float32)

    tokens = x.reshape(E, T, D)
    in_maps = []
    for e in range(E):
        xT = np.ascontiguousarray(
            tokens[e].reshape(NCHUNK, 512, DO, P).transpose(0, 3, 2, 1)
        ).astype(np.float16)  # [c, p, do, t']

        # Strassen B-side combos of X = tokens^T [D, T]
        X = tokens[e].T
        B11 = X[: D // 2, : T // 2]
        B12 = X[: D // 2, T // 2 :]
        B21 = X[D // 2 :, : T // 2]
        B22 = X[D // 2 :, T // 2 :]
        T1 = B12 - B11
        T2 = B22 - T1
        T3 = B22 - B12
        T4 = T2 - B21
        xS = np.ascontiguousarray(
            np.stack([T1, T2, T3, T4]).reshape(4, DH, P, 512).transpose(0, 2, 1, 3)
        ).astype(np.float16)  # [combo, p, k, t]

        # Strassen A-side combos of w1 (A = w1^T; combos computed in
        # w1-layout [d, f] since transposition is linear)
        w1e = w1[e]
        A11 = w1e[: D // 2, : F // 2]
        A12 = w1e[D // 2 :, : F // 2]
        A21 = w1e[: D // 2, F // 2 :]
        A22 = w1e[D // 2 :, F // 2 :]
        S1 = A21 + A22
        S2 = S1 - A11
        S3 = A11 - A21
        S4 = A12 - S2
        W1S = np.stack([A11, A12, S4, A22, S1, S2, S3])  # [7, D/2, F/2]
        # paired along the product order: groups (M1,M2) (M6,M7) (M5,M3),
        # M4 solo — [g, 2, D/2, F/2] -> [g, unit, p, 2, k, f]
        w1pairs = np.stack(
            [np.stack([W1S[a], W1S[b]]) for a, b in ((0, 1), (5, 6), (4, 2))]
        )
        w1p = np.ascontiguousarray(
            w1pairs.reshape(3, 2, DH, P, FH, P).transpose(0, 4, 3, 1, 2, 5)
        ).astype(np.float16)
        w1q = np.ascontiguousarray(
            W1S[3].reshape(DH, P, FH, P).transpose(2, 1, 0, 3)
        ).astype(np.float16)  # [unit, p(d), k, f]

        bc = np.ascontiguousarray(
            np.concatenate([b1[e].reshape(FT, P).T, b2[e].reshape(DMO, P).T], axis=1)
        )  # [p, ft..dmo]

        # Strassen A-side combos of w2 [F, D]
        w2e = w2[e]
        A11 = w2e[: F // 2, : D // 2]
        A12 = w2e[F // 2 :, : D // 2]
        A21 = w2e[: F // 2, D // 2 :]
        A22 = w2e[F // 2 :, D // 2 :]
        S1 = A21 + A22
        S2 = S1 - A11
        S3 = A11 - A21
        S4 = A12 - S2
        W2S = np.stack([A11, A12, S4, A22, S1, S2, S3])  # [7, F/2, D/2]
        w2s = np.ascontiguousarray(
            W2S.reshape(7, FH, P, 4, P).transpose(0, 3, 2, 1, 4)
        ).astype(np.float16)  # [mi, j, p, k, dm]
        in_maps.append(
            {"xT": xT, "xS": xS, "w1p": w1p, "w1q": w1q, "bc": bc, "w2s": w2s}
        )
    return in_maps


def gather_out(results):
    out = np.empty((E, T, D), dtype=np.float32)
    for e in range(E):
        yT = results[e]["yT"]  # [dmo, p, t]
        out[e] = yT.transpose(2, 0, 1).reshape(T, D)
    return out.reshape(B, S, D)


def kernel(x, w1, b1, w2, b2):
    from concourse.bass_utils import run_bass_kernel_spmd

    nc = _get_nc()
    in_maps = make_in_maps(x, w1, b1, w2, b2)
    res = run_bass_kernel_spmd(nc, in_maps, core_ids=list(range(E)))
    return gather_out(res.results)
